# revision 39
# baseline (speedup 1.0000x reference)
# Trainium2 Bass kernel for DirectSoftTreeEnsemble forward pass.
#
# Math (reference):
#   temp = clip(exp(log_temperature), 0.1, 5)
#   logits[b,t,i] = x[b,:] @ split_weights[t,i,:] + split_biases[t,i]      (i: 63 internal nodes)
#   s = sigmoid(logits / temp)
#   mu[b,t,l]     = prod over path of s / (1-s)                            (l: 64 leaves, depth 6)
#   P[t,l,:]      = softmax(leaf_logits[t,l,:] / temp)                     (C=1000 classes)
#   w             = softmax(tree_weights)                                  (T=32 trees)
#   out[b,c]      = sum_{t,l} mu[b,t,l] * w[t] * P[t,l,c]
#
# Strategy: data-parallel over batch (4096 -> 8 cores x 512 rows), tree params
# replicated.  Per core, two big matmuls on the PE array:
#   stage A: [512,1024] @ [1024,2048(ti,padded)]   (split logits)
#   stage B: [512,2048(tl)] @ [2048,1000]          (leaf blend)
# Glue on ACT (tanh/exp) and DVE (path-product doubling, softmax scale).
# sigmoid is computed via tanh so ACT needs only one function-table set:
#   2*s = 1 + tanh(z/(2*temp)),  2*(1-s) = 1 - tanh(z/(2*temp))
# The doubling then produces 64*mu; the 1/64 is folded into the leaf scale.
# The leaf softmax denominator Z comes free from the exp pass (accum_out).
#
# Host does only: sharding/layout/dtype prep, the 32-element tree softmax and
# scalar temperature value; all O(B*...)/O(T*L*C) math runs on device.

import numpy as np
import ml_dtypes

import concourse.bass as bass
import concourse.mybir as mybir
import concourse.tile as tile
from concourse import bacc
from concourse.bass_utils import run_bass_kernel_spmd

BF16 = mybir.dt.bfloat16
F32 = mybir.dt.float32
F32R = mybir.dt.float32r
FP8 = mybir.dt.float8e4
AF = mybir.ActivationFunctionType
OP = mybir.AluOpType

# Problem shapes (hardcoded per contract)
B, D, C, T, DEPTH = 4096, 1024, 1000, 32, 6
NI = 2**DEPTH - 1          # 63 internal nodes / tree
L = 2**DEPTH               # 64 leaves / tree
NIP = 64                   # padded internal nodes / tree
TIP = T * NIP              # 2048 padded internal total
TL = T * L                 # 2048 leaf rows total
NCORES = 8
BS = B // NCORES           # 512 batch rows / core
MT = BS // 128             # 4 m-tiles / core
KA = D // 128              # 8 k-tiles, stage A
KB = TL // 128             # 16 k-tiles, stage B
NB_CHUNKS = [(0, 512), (512, C - 512)]  # stage-B n chunks (512, 488)


def _build(a_fp32r: bool, has_bias: bool, unit_temp: bool):
    """Build the per-core SPMD Bass program."""
    nc = bacc.Bacc("TRN2", target_bir_lowering=False, debug=False)

    a_dt = F32 if a_fp32r else BF16
    xT_d = nc.dram_tensor("xT", [D, BS], a_dt, kind="ExternalInput")
    wT_d = nc.dram_tensor("wT", [D, TIP], a_dt, kind="ExternalInput")
    # ll3[p, s, :] = leaf row (s*128 + p); matches the DMA-transpose layout of mu^T
    # fp8: leaf logits are ~N(0, 0.1); quantization washes out in the softmax
    ll_d = nc.dram_tensor("ll", [128, KB, C], FP8, kind="ExternalInput")
    wm_d = nc.dram_tensor("wm", [128, KB], F32, kind="ExternalInput")
    out_d = nc.dram_tensor("out", [BS, C], F32, kind="ExternalOutput")
    if has_bias:
        bias_d = nc.dram_tensor("biasb", [128, TIP], F32, kind="ExternalInput")
    if not unit_temp:
        lt_d = nc.dram_tensor("lt", [1, 1], F32, kind="ExternalInput")

    with tile.TileContext(nc) as tc:
        consts = tc.alloc_tile_pool(name="consts", bufs=1)
        work = tc.alloc_tile_pool(name="work", bufs=2)
        psp = tc.alloc_tile_pool(name="psp", bufs=8, space="PSUM")

        # ---- temperature scalars -> per-partition [128,1] scale APs ----
        if unit_temp:
            ht_scale = 0.5       # tanh scale: 1/(2*temp)
            et_scale = 1.0       # exp scale: 1/temp
        else:
            ltb = consts.tile([128, 1], F32)
            nc.gpsimd.dma_start(out=ltb, in_=lt_d[:, :].partition_broadcast(128))
            tmp = consts.tile([128, 1], F32)
            nc.scalar.activation(tmp, ltb, AF.Exp)                  # temp
            nc.vector.tensor_scalar(tmp, tmp, 5.0, 0.1, OP.min, OP.max)
            itp = consts.tile([128, 1], F32)
            nc.vector.reciprocal(itp, tmp)                          # 1/temp
            htt = consts.tile([128, 1], F32)
            nc.vector.tensor_scalar_mul(htt, itp, 0.5)              # 1/(2 temp)
            ht_scale = htt[:, :]
            et_scale = itp[:, :]

        # ---- resident inputs, chunked + spread over two DMA queues so that
        # WT (stage A) and ll (exp) stream concurrently ----
        xTs = consts.tile([128, KA, BS], a_dt)
        wTs = consts.tile([128, KA, TIP], a_dt)
        wm = consts.tile([128, KB], F32)
        ll3 = consts.tile([128, KB, C], FP8)
        # Arrival-ordered across two DMA queues: first stage-A operands and the
        # fp8 leaf matrix early, later WT chunks stream behind.
        xT3 = xT_d[:, :].rearrange("(k p) b -> p k b", p=128)

        def dma_wt(k, eng):
            eng.dma_start(wTs[:, k, :], wT_d[k * 128:(k + 1) * 128, :])

        def dma_xt(j, eng):
            eng.dma_start(xTs[:, 2 * j:2 * j + 2, :], xT3[:, 2 * j:2 * j + 2, :])

        # gpsimd queue: stage-A operands in consumption order
        dma_wt(0, nc.gpsimd)
        dma_xt(0, nc.gpsimd)
        dma_wt(2, nc.gpsimd)
        dma_xt(1, nc.gpsimd)
        dma_wt(4, nc.gpsimd)
        dma_xt(2, nc.gpsimd)
        dma_wt(6, nc.gpsimd)
        dma_xt(3, nc.gpsimd)
        nc.gpsimd.dma_start(wm, wm_d[:, :])
        # SP queue: leaf logits first (feeds the exp chain), odd WT chunks after
        for g in range(4):
            nc.sync.dma_start(ll3[:, 4 * g:4 * (g + 1), :],
                              ll_d[:, 4 * g:4 * (g + 1), :])
        for k in range(1, KA, 2):
            dma_wt(k, nc.sync)
        if has_bias:
            biasb = consts.tile([128, TIP], F32)
            nc.sync.dma_start(biasb, bias_d[:, :])

        P3 = consts.tile([128, KB, C], BF16)
        Z = consts.tile([128, KB], F32)
        muT3 = consts.tile([128, KB, BS], BF16)  # mu^T, lhsT for stage B
        th_t, om_t = {}, {}

        def mm_dt(ap):
            return ap.bitcast(F32R) if a_fp32r else ap

        pa_t = {}

        def stage_a_half(m0, m1):
            # k-outer over an m-pair: 8 open PSUM groups track WT chunk arrival,
            # so the PE has runnable matmuls as soon as each k-chunk lands.
            for k in range(KA):
                for m in (m0, m1):
                    msl = slice(m * 128, (m + 1) * 128)
                    for n in range(4):
                        if k == 0:
                            pa_t[(m, n)] = psp.tile(
                                [128, 512], F32, name=f"pa{m}_{n}", tag="ps")
                        nc.tensor.matmul(
                            pa_t[(m, n)], mm_dt(xTs[:, k, msl]),
                            mm_dt(wTs[:, k, n * 512:(n + 1) * 512]),
                            start=(k == 0), stop=(k == KA - 1))

        def tanh_m(m):
            th = work.tile([128, TIP], BF16, name=f"th{m}", tag="th")
            th_t[m] = th
            for n in range(4):
                pa = pa_t[(m, n)]
                nsl = slice(n * 512, (n + 1) * 512)
                if has_bias:
                    nc.vector.tensor_tensor(pa, pa, biasb[:, nsl], OP.add)
                nc.scalar.activation(th[:, nsl], pa, AF.Tanh, scale=ht_scale)

        def doubling(m):
            # Path-product doubling, all bf16 (DVE 2x mode: step-1, 4B-aligned).
            # Node layout (host-permuted): within each 64-col tree block,
            # level d lives at cols [2^d, 2^(d+1)), in bit-reversed order so
            # the level-d node for LSB-first path j' sits at col 2^d + j'.
            th = th_t[m]
            omt = work.tile([128, TIP], BF16, name=f"omt{m}", tag="omt")
            nc.vector.tensor_scalar(omt, th, -1.0, 1.0, OP.mult, OP.add)
            opt = work.tile([128, TIP], BF16, name=f"opt{m}", tag="opt")
            nc.vector.tensor_scalar_add(opt, th, 1.0)
            th3 = th.rearrange("p (t i) -> p t i", t=T)
            om3 = omt.rearrange("p (t i) -> p t i", t=T)
            op3 = opt.rearrange("p (t i) -> p t i", t=T)
            muA = work.tile([128, T * 32], BF16, name=f"muA{m}", tag="muA")
            muB = work.tile([128, T * 32], BF16, name=f"muB{m}", tag="muB")
            mu6 = work.tile([128, TL], BF16, name=f"mu6{m}", tag="mu6")

            def lvl_view(d):
                # mu_d laid out [p, t, 2^d]; odd levels in muA, even in muB
                buf = muA if d % 2 == 1 else muB
                return buf[:, :T * (2 ** d)].rearrange("p (t j) -> p t j", t=T)

            # level 0 (root at col 1): mu1 = [1-th(root), 1+th(root)]
            mu1 = lvl_view(1)
            nc.vector.tensor_scalar_add(mu1[:, :, 0], om3[:, :, 1], 0.0)
            nc.vector.tensor_scalar_add(mu1[:, :, 1], th3[:, :, 1], 1.0)

            for d in range(1, DEPTH):
                lo, hi = 2 ** d, 2 ** (d + 1)
                mu_d = lvl_view(d)
                if d == DEPTH - 1:
                    dst = mu6.rearrange("p (t j) -> p t j", t=T)
                else:
                    dst = lvl_view(d + 1)
                half = 2 ** d
                # left children block: mu * (1 - th)
                nc.vector.tensor_tensor(
                    dst[:, :, :half], mu_d, om3[:, :, lo:hi], OP.mult)
                # right children block: mu * (1 + th)
                nc.vector.tensor_tensor(
                    dst[:, :, half:], mu_d, op3[:, :, lo:hi], OP.mult)
            # transpose mu (bf16): muT3[p, s, b] = mu6[b, s*128 + p]
            # (on the ACT hwdge queue: SP's queue is busy with input loads)
            nc.scalar.dma_start_transpose(muT3[:, :, m * 128:(m + 1) * 128],
                                          mu6[:, :])

        Zi = consts.tile([128, KB], F32)
        scl = consts.tile([128, KB], F32)

        def leaf_exp(s0, s1):
            # P = exp(ll/temp) * w_t/(64*Z); Z accumulated for free by ACT.
            # Per-s normalization so each P3 segment finalizes right after its exp.
            for s in range(s0, s1):
                nc.scalar.activation(P3[:, s, :], ll3[:, s, :], AF.Exp,
                                     scale=et_scale, accum_out=Z[:, s:s + 1])
                nc.vector.reciprocal(Zi[:, s:s + 1], Z[:, s:s + 1])
                nc.vector.tensor_tensor(scl[:, s:s + 1], Zi[:, s:s + 1],
                                        wm[:, s:s + 1], OP.mult)
                nc.vector.tensor_scalar_mul(P3[:, s, :], P3[:, s, :],
                                            scl[:, s:s + 1])

        def stage_b(m):
            msl = slice(m * 128, (m + 1) * 128)
            outm = work.tile([128, C], F32, name=f"outm{m}", tag="outm")
            for (c0, cn) in NB_CHUNKS:
                pb = psp.tile([128, 512], F32, name=f"pb{m}_{c0}", tag="ps")
                for k in range(KB):
                    nc.tensor.matmul(
                        pb[:, :cn], muT3[:, k, msl], P3[:, k, c0:c0 + cn],
                        start=(k == 0), stop=(k == KB - 1))
                nc.any.tensor_copy(outm[:, c0:c0 + cn], pb[:, :cn])
                nc.sync.dma_start(out_d[msl, c0:c0 + cn], outm[:, c0:c0 + cn])

        # Emission order shapes each engine's in-order program.
        leaf_exp(0, KB)
        stage_a_half(0, 1)
        tanh_m(0)
        tanh_m(1)
        doubling(0)
        doubling(1)
        stage_a_half(2, 3)
        tanh_m(2)
        tanh_m(3)
        doubling(2)
        doubling(3)
        for m in range(MT):
            stage_b(m)

        psp.release()
        work.release()
        consts.release()

    nc.compile()
    return nc


_cache = {}


def _get_nc(key):
    if key not in _cache:
        _cache[key] = _build(*key)
    return _cache[key]


A_FP32R = False  # stage-A matmul dtype lever (False = bf16)


def kernel(x, split_weights, split_biases, leaf_logits, tree_weights,
           log_temperature):
    x = np.asarray(x, np.float32)
    split_weights = np.asarray(split_weights, np.float32)
    split_biases = np.asarray(split_biases, np.float32)
    leaf_logits = np.asarray(leaf_logits, np.float32)
    tree_weights = np.asarray(tree_weights, np.float32)
    lt = float(np.asarray(log_temperature, np.float32).reshape(-1)[0])

    has_bias = bool(np.any(split_biases != 0.0))
    unit_temp = (lt == 0.0)
    a_fp32r = A_FP32R
    a_np = np.float32 if a_fp32r else ml_dtypes.bfloat16

    # ---- host layout prep ----
    # Node permutation: within each 64-col tree block, col 0 is padding and
    # level d occupies cols [2^d, 2^(d+1)) holding BFS node (2^d-1)+bitrev_d(r)
    # at col 2^d + r; leaves end up in LSB-first path order = bitrev6(BFS).
    def bitrev(v, bits):
        r = 0
        for _ in range(bits):
            r = (r << 1) | (v & 1)
            v >>= 1
        return r

    node_src = np.zeros(NIP, np.int64)  # padded col -> BFS node (col 0 -> pad)
    for d in range(DEPTH):
        for r in range(2 ** d):
            node_src[2 ** d + r] = (2 ** d - 1) + bitrev(r, d)
    leaf_src = np.array([bitrev(j, DEPTH) for j in range(L)], np.int64)

    # W^T [D, TIP]: permuted + padded node columns
    wpad = np.zeros((T, NIP, D), np.float32)
    wpad[:, 1:, :] = split_weights[:, node_src[1:], :]
    wT = np.ascontiguousarray(wpad.reshape(TIP, D).T.astype(a_np))
    # x^T shards [D, BS] per core
    xT = x.T.astype(a_np)
    xT_shards = [np.ascontiguousarray(xT[:, c * BS:(c + 1) * BS])
                 for c in range(NCORES)]
    # leaf logits: bitrev leaf order, then [TL, C] -> [128, KB, C] with
    # ll3[p, s, :] = permuted row s*128+p
    ll_perm = leaf_logits[:, leaf_src, :].reshape(TL, C)
    ll = np.ascontiguousarray(
        ll_perm.reshape(KB, 128, C).transpose(1, 0, 2)
        .astype(ml_dtypes.float8_e4m3))
    # tree-weight softmax (32 scalars on host) folded with the 1/64 doubling fixup
    twf = tree_weights - tree_weights.max()
    w = np.exp(twf) / np.exp(twf).sum()
    w64 = (w / 64.0).astype(np.float32)
    # wm[p, s] = w[(s*128+p)//64] / 64
    p_idx = np.arange(128)[:, None]
    s_idx = np.arange(KB)[None, :]
    wm = np.ascontiguousarray(w64[(s_idx * 128 + p_idx) // 64])

    in_map_common = {"wT": wT, "ll": ll, "wm": wm}
    if has_bias:
        bpad = np.zeros((T, NIP), np.float32)
        bpad[:, 1:] = split_biases[:, node_src[1:]]
        in_map_common["biasb"] = np.ascontiguousarray(
            np.broadcast_to(bpad.reshape(1, TIP), (128, TIP)).astype(np.float32))
    if not unit_temp:
        in_map_common["lt"] = np.full((1, 1), lt, np.float32)

    nc = _get_nc((a_fp32r, has_bias, unit_temp))
    in_maps = [{"xT": xT_shards[c], **in_map_common} for c in range(NCORES)]
    res = run_bass_kernel_spmd(nc, in_maps, core_ids=list(range(NCORES)))
    global LAST_RESULT
    LAST_RESULT = res
    out = np.concatenate([r["out"] for r in res.results], axis=0)
    return np.ascontiguousarray(out.astype(np.float32))


LAST_RESULT = None
